# revision 15
# baseline (speedup 1.0000x reference)
"""MQA kernel for Trainium2 (8 NeuronCores, SPMD via bass/Tile).

Problem: nn_MultiQueryAttention (B=2, T=2048, HID=2048, H=16, D=128).

Key algebraic simplification: the reference's apply_rope treats q's layout
as (B,T,H,D) while q is actually (B,H,T,D), so the "position" axis is the
head index -> per-head rotation R_h acting on the D axis only, independent
of sequence position. R_h is folded into Wq on the host. k's rope at pos=0
is a pure channel permutation, folded into Wk. The score scale 1/sqrt(D)
is folded into Wq as well. What remains on-device is a plain causal MQA.

Sharding (uniform SPMD program, per-core data differs):
  core c -> batch c//4, heads (c%4)*4..(c%4)*4+3, full T.
  Each core: Q^T/K^T/V projections, causal softmax attention for its 4
  heads, and a partial out-projection (its heads' rows of Wo^T). The 4
  partials per batch are summed on the host.

v2 changes vs the first working version:
  * all matmul operands in bf16 (same PE rate as fp32r, half the DMA and
    half the LDWEIGHTS traffic via FWL); PSUM accumulation stays fp32.
  * phase 1 restructured output-block-major with 2 rotating PSUM banks
    (no more full-PE stalls at PSUM drain), V projected directly in its
    natural [s,d] layout (hst-slice stationary), no PE transposes.
  * softmax denominator matmul uses a full-ones stationary matrix so the
    denominator lands pre-broadcast across all 128 partitions: the
    recip-broadcast matmul and two DVE copies disappear from the per-row
    critical chain.
  * diagonal (masked) key tile is processed first in each query row so
    the mask multiply is off the critical path.
  * out-projection of row tb-1 is emitted inside row tb, right after the
    first scores matmul, to hide the exp/recip latency chain.
  * few large host-pre-laid-out DMAs instead of ~180 small ones.
"""

import numpy as np
import ml_dtypes
from contextlib import ExitStack

import concourse.bass as bass
import concourse.tile as tile
from concourse import bacc, mybir
from concourse.bass_utils import run_bass_kernel_spmd

F32 = mybir.dt.float32
BF16 = mybir.dt.bfloat16
EXP = mybir.ActivationFunctionType.Exp
NPBF = ml_dtypes.bfloat16

B, T, HID, H, D = 2, 2048, 2048, 16, 128
NCORES = 8
CPB = 4              # cores per batch
HPC = H // CPB       # 4 heads per core
HD_PC = HPC * D      # 512 output dims per core
P = 128
KT = T // P          # 16 key tiles
NK = HID // P        # 16 contraction tiles for projections
TB4 = T // 512       # 4 phase-1 t blocks


def _rope_fold():
    """Per-head rotation matrices R_h (128x128) from the reference's quirky rope."""
    half = D // 2
    theta = 1.0 / (10000.0 ** (np.arange(0, half, 2, dtype=np.float64) / half))
    mats = []
    for h in range(H):
        R = np.zeros((D, D), dtype=np.float64)
        c = np.cos(h * theta)
        s = np.sin(h * theta)
        for j in range(32):
            R[j, 2 * j] = c[j]
            R[j, 2 * j + 1] = -s[j]
            R[32 + j, 2 * j] = s[j]
            R[32 + j, 2 * j + 1] = c[j]
            R[64 + j, 64 + 2 * j] = c[j]
            R[64 + j, 64 + 2 * j + 1] = -s[j]
            R[96 + j, 64 + 2 * j] = s[j]
            R[96 + j, 64 + 2 * j + 1] = c[j]
        mats.append(R)
    return mats


def _build_program():
    nc = bacc.Bacc("TRN2", target_bir_lowering=False, debug=False,
                   enable_asserts=False, num_devices=NCORES)

    hst_d = nc.dram_tensor("hst", [TB4, P, NK, 512], BF16, kind="ExternalInput").ap()
    hkv_d = nc.dram_tensor("hkv", [P, NK, 512], BF16, kind="ExternalInput").ap()
    wq_d = nc.dram_tensor("wq", [P, NK, HD_PC], BF16, kind="ExternalInput").ap()
    wk_d = nc.dram_tensor("wk", [P, NK, D], BF16, kind="ExternalInput").ap()
    wv_d = nc.dram_tensor("wv", [P, NK, D], BF16, kind="ExternalInput").ap()
    wo_d = nc.dram_tensor("wo", [P, HPC, HID], BF16, kind="ExternalInput").ap()
    msk_d = nc.dram_tensor("dmask", [P, P], BF16, kind="ExternalInput").ap()
    ones_d = nc.dram_tensor("onesd", [P, P], BF16, kind="ExternalInput").ap()
    out_d = nc.dram_tensor("out", [KT, P, HID], BF16, kind="ExternalOutput").ap()

    def mm(ps, lhsT, rhs, start, stop):
        nc.tensor.matmul(ps, lhsT=lhsT, rhs=rhs, start=start, stop=stop)

    with tile.TileContext(nc) as tc, ExitStack() as ctx:
        singles = ctx.enter_context(tc.tile_pool(name="singles", bufs=1))
        epool = ctx.enter_context(tc.tile_pool(name="etile", bufs=6))
        rpool = ctx.enter_context(tc.tile_pool(name="small", bufs=2))
        opool = ctx.enter_context(tc.tile_pool(name="outt", bufs=2))

        msk = singles.tile([P, P], BF16)
        ones = singles.tile([P, P], BF16)

        wq_sb = singles.tile([P, NK, HD_PC], BF16)
        wk_sb = singles.tile([P, NK, D], BF16)
        wv_sb = singles.tile([P, NK, D], BF16)
        wo_sb = singles.tile([P, HPC, HID], BF16)
        hst_sb = singles.tile([P, TB4, NK, 512], BF16)
        hkv_sb = singles.tile([P, NK, 512], BF16)
        kvstage = singles.tile([P, 1024], BF16)

        qt_sb = singles.tile([P, HPC, T], BF16)      # Q^T per head [d, t]
        kt_sb = singles.tile([P, T], BF16)           # K^T [d, s]
        v_sb = singles.tile([P, KT, D], BF16)        # V natural [s-tile, d]
        at_sb = singles.tile([P, HPC, T], BF16)      # normalized O^T per head

        # ---- DMAs, in consumption order (few, large descriptors) ----
        nc.sync.dma_start(out=wk_sb, in_=wk_d)
        nc.sync.dma_start(out=hkv_sb, in_=hkv_d)
        nc.sync.dma_start(out=wv_sb, in_=wv_d)
        for kq in range(4):
            ksl = slice(kq * 4, (kq + 1) * 4)
            nc.sync.dma_start(out=wq_sb[:, ksl, :], in_=wq_d[:, ksl, :])
            nc.sync.dma_start(out=hst_sb[:, 0, ksl, :], in_=hst_d[0][:, ksl, :])
        for tb4 in range(1, TB4):
            nc.sync.dma_start(out=hst_sb[:, tb4, :, :], in_=hst_d[tb4])
        nc.sync.dma_start(out=msk, in_=msk_d)
        nc.sync.dma_start(out=ones, in_=ones_d)
        for h in range(HPC):
            nc.sync.dma_start(out=wo_sb[:, h, :], in_=wo_d[:, h, :])

        # ---- K/V projections: this core's T/4 slice only, then AllGather
        # across the 4 cores of the batch group (classic MQA dedup: the
        # shared K/V head is otherwise computed 4x). The quarter compute
        # also fills the DMA-starved kernel start.
        with tc.tile_pool(name="dramkv", bufs=1, space="DRAM") as dramkv, \
             tc.tile_pool(name="pskv", bufs=1, space="PSUM") as pskv:
            kv_in = dramkv.tile([P, 1024], BF16)
            kv_out = dramkv.tile([CPB, P, 1024], BF16)
            kz = pskv.tile([P, 512], F32, tag="kz")
            for k in range(NK):
                mm(kz[:], wk_sb[:, k, :], hkv_sb[:, k, :], k == 0, k == NK - 1)
            nc.vector.tensor_copy(kvstage[:, 0:512], kz[:])
            vq = pskv.tile([P, 4, D], F32, tag="vq")
            for ti in range(4):
                for k in range(NK):
                    mm(vq[:, ti, :], hkv_sb[:, k, ti * P:(ti + 1) * P],
                       wv_sb[:, k, :], k == 0, k == NK - 1)
            nc.vector.tensor_copy(
                kvstage[:, 512:1024].rearrange("p (a b) -> p a b", a=4), vq[:])
            nc.gpsimd.dma_start(kv_in[:], kvstage[:])
            nc.gpsimd.collective_compute(
                "AllGather",
                mybir.AluOpType.bypass,
                replica_groups=[[0, 1, 2, 3], [4, 5, 6, 7]],
                ins=[kv_in[:].opt()],
                outs=[kv_out[:].opt()],
            )
            kvo = kv_out[:].rearrange("r p c -> p r c")
            nc.gpsimd.dma_start(
                kt_sb[:].rearrange("p (r s) -> p r s", r=CPB), kvo[:, :, 0:512])
            nc.gpsimd.dma_start(
                v_sb[:].rearrange("p (r a) b -> p r a b", r=CPB),
                kvo[:, :, 512:1024].rearrange("p r (a b) -> p r a b", a=4))

        # ---------------- Phase 1: Q/K/V projections ----------------
        with tc.tile_pool(name="ps1z", bufs=1, space="PSUM") as ps1z, \
             tc.tile_pool(name="ps1", bufs=2, space="PSUM") as ps1:
            # tb4 = 0: k-quad-blocked over four separate PSUM tiles (one per
            # q head) so compute tracks the DMA arrival order instead of
            # stalling for the whole 2 MB of weights+activations.
            zps = [ps1z.tile([P, 512], F32, tag=f"z{ob}", name=f"z{ob}")
                   for ob in range(HPC)]
            for kq in range(4):
                for ob in range(HPC):
                    for k in range(kq * 4, kq * 4 + 4):
                        mm(zps[ob][:], wq_sb[:, k, ob * D:(ob + 1) * D],
                           hst_sb[:, 0, k, :], k == 0, k == NK - 1)
            for ob in range(HPC):
                nc.vector.tensor_copy(qt_sb[:, ob, 0:512], zps[ob][:])

            for tb4 in range(1, TB4):
                tsl = slice(tb4 * 512, (tb4 + 1) * 512)
                for ob in range(HPC):
                    ps = ps1.tile([P, 512], F32, tag="proj")
                    for k in range(NK):
                        mm(ps[:], wq_sb[:, k, ob * D:(ob + 1) * D],
                           hst_sb[:, tb4, k, :], k == 0, k == NK - 1)
                    nc.vector.tensor_copy(qt_sb[:, ob, tsl], ps[:])

        # -------- Phase 2+3: causal attention + fused out-projection ----
        msk_b = msk[:, None, :].to_broadcast([P, HPC, P])

        with tc.tile_pool(name="ps2s", bufs=3, space="PSUM") as ps2s, \
             tc.tile_pool(name="ps2o", bufs=2, space="PSUM") as ps2o, \
             tc.tile_pool(name="ps2d", bufs=1, space="PSUM") as ps2d, \
             tc.tile_pool(name="ps3", bufs=2, space="PSUM") as ps3:

            def op_chunk(tb, jb):
                # one quarter of row tb's out-projection + cast + DMA out
                tsl = slice(tb * P, (tb + 1) * P)
                jsl = slice(jb * 512, (jb + 1) * 512)
                if jb == 0:
                    op_chunk.ost = opool.tile([P, HID], BF16, tag="ost")
                op_ps = ps3.tile([P, 512], F32, tag="op")
                for h in range(HPC):
                    mm(op_ps[:], at_sb[:, h, tsl], wo_sb[:, h, jsl],
                       h == 0, h == HPC - 1)
                nc.vector.tensor_copy(op_chunk.ost[:, jsl], op_ps[:])
                nc.sync.dma_start(out=out_d[tb][:, jsl], in_=op_chunk.ost[:, jsl])

            # Mostly-descending row order with the tiny rows (few key tiles)
            # interspersed mid-stream: their recip chains hide under big
            # rows, and the kernel ends on a medium row instead of a chain
            # of tiny rows with nothing left to overlap.
            rows = [15, 14, 13, 12, 0, 11, 1, 10, 2, 9, 3, 8, 7, 6, 5, 4]
            for ri, tb in enumerate(rows):
                tsl = slice(tb * P, (tb + 1) * P)
                qrhs = qt_sb[:, :, tsl]              # [128, 4, 128]
                den_ps = ps2d.tile([P, HPC, P], F32, tag="den")
                ot_ps = ps2o.tile([P, HPC, P], F32, tag="ot")
                sts = [tb] + list(range(tb))         # diagonal tile first
                n = len(sts)
                e_tiles = {}

                def sc_exp(j):
                    s_ps = ps2s.tile([P, HPC, P], F32, tag="sps")
                    mm(s_ps[:], kt_sb[:, sts[j] * P:(sts[j] + 1) * P], qrhs,
                       True, True)
                    e_sb = epool.tile([P, HPC, P], BF16, tag="etile")
                    nc.scalar.activation(e_sb[:], s_ps[:], EXP)
                    if j == 0:  # diagonal tile: causal mask
                        nc.vector.tensor_mul(e_sb[:], e_sb[:], msk_b)
                    e_tiles[j] = e_sb

                # 2-deep software pipeline: scores/exp run 2 pairs ahead of
                # den/pv so the exp latency never stalls the PE.
                opq = [] if ri == 0 else [0, 1, 2, 3]
                prev_tb = rows[ri - 1] if ri > 0 else None
                for j in range(min(2, n)):
                    sc_exp(j)
                for i in range(n):
                    if i + 2 < n:
                        sc_exp(i + 2)
                    e_sb = e_tiles.pop(i)
                    st_, sp_ = (i == 0), (i == n - 1)
                    mm(den_ps[:], ones[:, :], e_sb[:], st_, sp_)
                    mm(ot_ps[:], v_sb[:, sts[i], :], e_sb[:], st_, sp_)
                    if opq and 2 <= i:
                        op_chunk(prev_tb, opq.pop(0))
                while opq:
                    op_chunk(prev_tb, opq.pop(0))
                recip = rpool.tile([P, HPC, P], F32, tag="recip")
                nc.vector.reciprocal_approx_fast(out=recip[:], in_=den_ps[:])
                nc.vector.tensor_mul(at_sb[:, :, tsl], ot_ps[:], recip[:])
            for jb in range(4):
                op_chunk(rows[-1], jb)

    nc.compile()
    return nc


_CACHE = {}


def _get_program():
    if "nc" not in _CACHE:
        _CACHE["nc"] = _build_program()
    return _CACHE["nc"]


def _host_inputs(hidden_states, Wq, Wk, Wv, Wo):
    """Fold rope+scale into weights, build per-core bf16 input maps."""
    f64 = np.float64
    mats = _rope_fold()
    scale = D ** -0.5
    Wq_f = np.empty((HID, HID), dtype=np.float32)
    for h in range(H):
        Wq_f[h * D:(h + 1) * D] = (mats[h] @ Wq[h * D:(h + 1) * D].astype(f64)
                                   * scale).astype(np.float32)
    perm = np.concatenate([np.arange(0, 64, 2), np.arange(1, 64, 2),
                           np.arange(64, 128, 2), np.arange(65, 128, 2)])
    Wk_f = Wk[perm].astype(np.float32)

    # [p, k, d] layouts: w_d[p, k, d] = W^T[k*128+p, d]
    wk_h = np.ascontiguousarray(
        Wk_f.T.reshape(NK, P, D).astype(NPBF))
    wk_h = np.ascontiguousarray(wk_h.transpose(1, 0, 2))
    wv_h = np.ascontiguousarray(
        Wv.astype(np.float32).T.reshape(NK, P, D).astype(NPBF).transpose(1, 0, 2))

    ii = np.arange(P)[:, None]
    jj = np.arange(P)[None, :]
    dmask = (ii <= jj).astype(NPBF)
    ones = np.ones((P, P), dtype=NPBF)

    # hst_d[tb4, p, k, t] = hs[b][tb4*512+t, k*128+p]
    hst_b = []
    for b in range(B):
        a = hidden_states[b].reshape(TB4, 512, NK, P).transpose(0, 3, 2, 1)
        hst_b.append(np.ascontiguousarray(a.astype(NPBF)))

    wq_q, wo_q = [], []
    for q in range(CPB):
        rows = slice(q * HD_PC, (q + 1) * HD_PC)
        # wq_d[p, k, hd] = Wq_f[q*512+hd, k*128+p]
        a = Wq_f[rows].T.reshape(NK, P, HD_PC).transpose(1, 0, 2)
        wq_q.append(np.ascontiguousarray(a.astype(NPBF)))
        # wo_d[p, h, n] = Wo[n, q*512 + h*128 + p]
        a = Wo[:, rows].T.reshape(HPC, P, HID).transpose(1, 0, 2)
        wo_q.append(np.ascontiguousarray(a.astype(NPBF)))

    in_maps = []
    for c in range(NCORES):
        b, q = c // CPB, c % CPB
        in_maps.append({
            "hst": hst_b[b],
            "hkv": np.ascontiguousarray(hst_b[b][q]),
            "wq": wq_q[q],
            "wk": wk_h,
            "wv": wv_h,
            "wo": wo_q[q],
            "dmask": dmask,
            "onesd": ones,
        })
    return in_maps


def kernel(hidden_states, Wq, Wk, Wv, Wo):
    hidden_states = np.asarray(hidden_states, dtype=np.float32)
    Wq = np.asarray(Wq, dtype=np.float32)
    Wk = np.asarray(Wk, dtype=np.float32)
    Wv = np.asarray(Wv, dtype=np.float32)
    Wo = np.asarray(Wo, dtype=np.float32)

    nc = _get_program()
    in_maps = _host_inputs(hidden_states, Wq, Wk, Wv, Wo)
    res = run_bass_kernel_spmd(nc, in_maps, list(range(NCORES)))
    parts = [r["out"].astype(np.float32).reshape(T, HID) for r in res.results]
    out = np.empty((B, T, HID), dtype=np.float32)
    for b in range(B):
        out[b] = parts[CPB * b]
        for q in range(1, CPB):
            out[b] += parts[CPB * b + q]
    return out


# revision 18
# speedup vs baseline: 1.1130x; 1.1130x over previous
"""MQA kernel for Trainium2 (8 NeuronCores, SPMD via bass/Tile).

Problem: nn_MultiQueryAttention (B=2, T=2048, HID=2048, H=16, D=128).

Key algebraic simplification: the reference's apply_rope treats q's layout
as (B,T,H,D) while q is actually (B,H,T,D), so the "position" axis is the
head index -> per-head rotation R_h acting on the D axis only, independent
of sequence position. R_h is folded into Wq on the host. k's rope at pos=0
is a pure channel permutation, folded into Wk. The score scale 1/sqrt(D)
is folded into Wq as well. What remains on-device is a plain causal MQA.

Sharding (uniform SPMD program, per-core data differs):
  core c -> batch c//4, heads (c%4)*4..(c%4)*4+3, full T.
  Each core: Q^T/K^T/V projections, causal softmax attention for its 4
  heads, and a partial out-projection (its heads' rows of Wo^T). The 4
  partials per batch are summed on the host.

v2 changes vs the first working version:
  * all matmul operands in bf16 (same PE rate as fp32r, half the DMA and
    half the LDWEIGHTS traffic via FWL); PSUM accumulation stays fp32.
  * phase 1 restructured output-block-major with 2 rotating PSUM banks
    (no more full-PE stalls at PSUM drain), V projected directly in its
    natural [s,d] layout (hst-slice stationary), no PE transposes.
  * softmax denominator matmul uses a full-ones stationary matrix so the
    denominator lands pre-broadcast across all 128 partitions: the
    recip-broadcast matmul and two DVE copies disappear from the per-row
    critical chain.
  * diagonal (masked) key tile is processed first in each query row so
    the mask multiply is off the critical path.
  * out-projection of row tb-1 is emitted inside row tb, right after the
    first scores matmul, to hide the exp/recip latency chain.
  * few large host-pre-laid-out DMAs instead of ~180 small ones.
"""

import numpy as np
import ml_dtypes
from contextlib import ExitStack

import concourse.bass as bass
import concourse.tile as tile
from concourse import bacc, mybir
from concourse.bass_utils import run_bass_kernel_spmd

F32 = mybir.dt.float32
BF16 = mybir.dt.bfloat16
EXP = mybir.ActivationFunctionType.Exp
NPBF = ml_dtypes.bfloat16

B, T, HID, H, D = 2, 2048, 2048, 16, 128
NCORES = 8
CPB = 4              # cores per batch
HPC = H // CPB       # 4 heads per core
HD_PC = HPC * D      # 512 output dims per core
P = 128
KT = T // P          # 16 key tiles
NK = HID // P        # 16 contraction tiles for projections
TB4 = T // 512       # 4 phase-1 t blocks


def _rope_fold():
    """Per-head rotation matrices R_h (128x128) from the reference's quirky rope."""
    half = D // 2
    theta = 1.0 / (10000.0 ** (np.arange(0, half, 2, dtype=np.float64) / half))
    mats = []
    for h in range(H):
        R = np.zeros((D, D), dtype=np.float64)
        c = np.cos(h * theta)
        s = np.sin(h * theta)
        for j in range(32):
            R[j, 2 * j] = c[j]
            R[j, 2 * j + 1] = -s[j]
            R[32 + j, 2 * j] = s[j]
            R[32 + j, 2 * j + 1] = c[j]
            R[64 + j, 64 + 2 * j] = c[j]
            R[64 + j, 64 + 2 * j + 1] = -s[j]
            R[96 + j, 64 + 2 * j] = s[j]
            R[96 + j, 64 + 2 * j + 1] = c[j]
        mats.append(R)
    return mats


def _build_program():
    nc = bacc.Bacc("TRN2", target_bir_lowering=False, debug=False,
                   enable_asserts=False, num_devices=NCORES)

    hst_d = nc.dram_tensor("hst", [TB4, P, NK, 512], BF16, kind="ExternalInput").ap()
    wq_d = nc.dram_tensor("wq", [P, NK, HD_PC], BF16, kind="ExternalInput").ap()
    wk_d = nc.dram_tensor("wk", [P, NK, D], BF16, kind="ExternalInput").ap()
    wv_d = nc.dram_tensor("wv", [P, NK, D], BF16, kind="ExternalInput").ap()
    wo_d = nc.dram_tensor("wo", [P, HPC, HID], BF16, kind="ExternalInput").ap()
    msk_d = nc.dram_tensor("dmask", [P, P], BF16, kind="ExternalInput").ap()
    ones_d = nc.dram_tensor("onesd", [P, P], BF16, kind="ExternalInput").ap()
    out_d = nc.dram_tensor("out", [KT, P, HID], BF16, kind="ExternalOutput").ap()

    def mm(ps, lhsT, rhs, start, stop):
        nc.tensor.matmul(ps, lhsT=lhsT, rhs=rhs, start=start, stop=stop)

    with tile.TileContext(nc) as tc, ExitStack() as ctx:
        singles = ctx.enter_context(tc.tile_pool(name="singles", bufs=1))
        epool = ctx.enter_context(tc.tile_pool(name="etile", bufs=8))
        rpool = ctx.enter_context(tc.tile_pool(name="small", bufs=2))
        opool = ctx.enter_context(tc.tile_pool(name="outt", bufs=2))

        msk = singles.tile([P, P], BF16)
        ones = singles.tile([P, P], BF16)

        wq_sb = singles.tile([P, NK, HD_PC], BF16)
        wk_sb = singles.tile([P, NK, D], BF16)
        wv_sb = singles.tile([P, NK, D], BF16)
        wo_sb = singles.tile([P, HPC, HID], BF16)
        hst_sb = singles.tile([P, TB4, NK, 512], BF16)

        qt_sb = singles.tile([P, HPC, T], BF16)      # Q^T per head [d, t]
        kt_sb = singles.tile([P, T], BF16)           # K^T [d, s]
        v_sb = singles.tile([P, KT, D], BF16)        # V natural [s-tile, d]
        at_sb = singles.tile([P, HPC, T], BF16)      # normalized O^T per head

        # ---- DMAs, in consumption order (few, large descriptors) ----
        for kq in range(4):
            ksl = slice(kq * 4, (kq + 1) * 4)
            nc.sync.dma_start(out=wq_sb[:, ksl, :], in_=wq_d[:, ksl, :])
            nc.sync.dma_start(out=hst_sb[:, 0, ksl, :], in_=hst_d[0][:, ksl, :])
            nc.sync.dma_start(out=wk_sb[:, ksl, :], in_=wk_d[:, ksl, :])
            nc.sync.dma_start(out=wv_sb[:, ksl, :], in_=wv_d[:, ksl, :])
        for tb4 in range(1, TB4):
            nc.sync.dma_start(out=hst_sb[:, tb4, :, :], in_=hst_d[tb4])
        nc.sync.dma_start(out=msk, in_=msk_d)
        nc.sync.dma_start(out=ones, in_=ones_d)
        for h in range(HPC):
            nc.sync.dma_start(out=wo_sb[:, h, :], in_=wo_d[:, h, :])

        # ---------------- Phase 1: Q/K/V projections ----------------
        with tc.tile_pool(name="ps1z", bufs=1, space="PSUM") as ps1z, \
             tc.tile_pool(name="ps1", bufs=2, space="PSUM") as ps1, \
             tc.tile_pool(name="ps1v", bufs=1, space="PSUM") as ps1v:
            # tb4 = 0: k-quad-blocked over five separate PSUM tiles (one per
            # output block) so compute tracks the DMA arrival order instead
            # of stalling for the whole 2.5 MB of weights+activations.
            zps = [ps1z.tile([P, 512], F32, tag=f"z{ob}", name=f"z{ob}")
                   for ob in range(5)]
            for kq in range(4):
                for ob in range(5):
                    for k in range(kq * 4, kq * 4 + 4):
                        if ob < HPC:
                            lhsT = wq_sb[:, k, ob * D:(ob + 1) * D]
                        else:
                            lhsT = wk_sb[:, k, :]
                        mm(zps[ob][:], lhsT, hst_sb[:, 0, k, :],
                           k == 0, k == NK - 1)
            for ob in range(HPC):
                nc.vector.tensor_copy(qt_sb[:, ob, 0:512], zps[ob][:])
            nc.vector.tensor_copy(kt_sb[:, 0:512], zps[HPC][:])
            vps = ps1v.tile([P, 4, D], F32, tag="vnat")
            for ti in range(4):
                for k in range(NK):
                    mm(vps[:, ti, :], hst_sb[:, 0, k, ti * P:(ti + 1) * P],
                       wv_sb[:, k, :], k == 0, k == NK - 1)
            nc.vector.tensor_copy(v_sb[:, 0:4, :], vps[:])

            for tb4 in range(1, TB4):
                tsl = slice(tb4 * 512, (tb4 + 1) * 512)
                for ob in range(5):          # q heads 0..3, then k
                    ps = ps1.tile([P, 512], F32, tag="proj")
                    for k in range(NK):
                        if ob < HPC:
                            lhsT = wq_sb[:, k, ob * D:(ob + 1) * D]
                        else:
                            lhsT = wk_sb[:, k, :]
                        mm(ps[:], lhsT, hst_sb[:, tb4, k, :], k == 0, k == NK - 1)
                    if ob < HPC:
                        nc.vector.tensor_copy(qt_sb[:, ob, tsl], ps[:])
                    else:
                        nc.vector.tensor_copy(kt_sb[:, tsl], ps[:])
                # V in natural [t, d] layout: hst-slice stationary
                vps = ps1v.tile([P, 4, D], F32, tag="vnat")
                for ti in range(4):
                    for k in range(NK):
                        mm(vps[:, ti, :],
                           hst_sb[:, tb4, k, ti * P:(ti + 1) * P],
                           wv_sb[:, k, :], k == 0, k == NK - 1)
                nc.vector.tensor_copy(v_sb[:, tb4 * 4:(tb4 + 1) * 4, :], vps[:])

        # -------- Phase 2+3: causal attention + fused out-projection ----
        msk_b = msk[:, None, :].to_broadcast([P, HPC, P])

        with tc.tile_pool(name="ps2s", bufs=3, space="PSUM") as ps2s, \
             tc.tile_pool(name="ps2o", bufs=2, space="PSUM") as ps2o, \
             tc.tile_pool(name="ps2d", bufs=1, space="PSUM") as ps2d, \
             tc.tile_pool(name="ps3", bufs=2, space="PSUM") as ps3:

            def op_chunk(tb, jb):
                # one quarter of row tb's out-projection + cast + DMA out
                tsl = slice(tb * P, (tb + 1) * P)
                jsl = slice(jb * 512, (jb + 1) * 512)
                if jb == 0:
                    op_chunk.ost = opool.tile([P, HID], BF16, tag="ost")
                op_ps = ps3.tile([P, 512], F32, tag="op")
                for h in range(HPC):
                    mm(op_ps[:], at_sb[:, h, tsl], wo_sb[:, h, jsl],
                       h == 0, h == HPC - 1)
                nc.vector.tensor_copy(op_chunk.ost[:, jsl], op_ps[:])
                nc.sync.dma_start(out=out_d[tb][:, jsl], in_=op_chunk.ost[:, jsl])

            # Mostly-descending row order with the tiny rows (few key tiles)
            # interspersed mid-stream: their recip chains hide under big
            # rows, and the kernel ends on a medium row instead of a chain
            # of tiny rows with nothing left to overlap.
            rows = [15, 14, 13, 12, 0, 11, 1, 10, 2, 9, 3, 8, 7, 6, 5, 4]
            for ri, tb in enumerate(rows):
                tsl = slice(tb * P, (tb + 1) * P)
                qrhs = qt_sb[:, :, tsl]              # [128, 4, 128]
                den_ps = ps2d.tile([P, HPC, P], F32, tag="den")
                ot_ps = ps2o.tile([P, HPC, P], F32, tag="ot")
                sts = [tb] + list(range(tb))         # diagonal tile first
                n = len(sts)
                e_tiles = {}

                def sc_exp(j):
                    s_ps = ps2s.tile([P, HPC, P], F32, tag="sps")
                    mm(s_ps[:], kt_sb[:, sts[j] * P:(sts[j] + 1) * P], qrhs,
                       True, True)
                    e_sb = epool.tile([P, HPC, P], BF16, tag="etile")
                    nc.scalar.activation(e_sb[:], s_ps[:], EXP)
                    if j == 0:  # diagonal tile: causal mask
                        nc.vector.tensor_mul(e_sb[:], e_sb[:], msk_b)
                    e_tiles[j] = e_sb

                # 2-deep software pipeline: scores/exp run 2 pairs ahead of
                # den/pv so the exp latency never stalls the PE.
                opq = [] if ri == 0 else [0, 1, 2, 3]
                prev_tb = rows[ri - 1] if ri > 0 else None
                for j in range(min(2, n)):
                    sc_exp(j)
                for i in range(n):
                    if i + 2 < n:
                        sc_exp(i + 2)
                    e_sb = e_tiles.pop(i)
                    st_, sp_ = (i == 0), (i == n - 1)
                    if i == 0:
                        # pv first: den bank (bufs=1) may still be read by the
                        # previous row's reciprocal — buy it one matmul of slack
                        mm(ot_ps[:], v_sb[:, sts[i], :], e_sb[:], st_, sp_)
                        mm(den_ps[:], ones[:, :], e_sb[:], st_, sp_)
                    else:
                        mm(den_ps[:], ones[:, :], e_sb[:], st_, sp_)
                        mm(ot_ps[:], v_sb[:, sts[i], :], e_sb[:], st_, sp_)
                    if opq and 2 <= i:
                        op_chunk(prev_tb, opq.pop(0))
                while opq:
                    op_chunk(prev_tb, opq.pop(0))
                recip = rpool.tile([P, HPC, P], F32, tag="recip")
                nc.vector.reciprocal_approx_fast(out=recip[:], in_=den_ps[:])
                nc.vector.tensor_mul(at_sb[:, :, tsl], ot_ps[:], recip[:])
            for jb in range(4):
                op_chunk(rows[-1], jb)

    nc.compile()
    return nc


_CACHE = {}


def _get_program():
    if "nc" not in _CACHE:
        _CACHE["nc"] = _build_program()
    return _CACHE["nc"]


def _host_inputs(hidden_states, Wq, Wk, Wv, Wo):
    """Fold rope+scale into weights, build per-core bf16 input maps."""
    f64 = np.float64
    mats = _rope_fold()
    scale = D ** -0.5
    Wq_f = np.empty((HID, HID), dtype=np.float32)
    for h in range(H):
        Wq_f[h * D:(h + 1) * D] = (mats[h] @ Wq[h * D:(h + 1) * D].astype(f64)
                                   * scale).astype(np.float32)
    perm = np.concatenate([np.arange(0, 64, 2), np.arange(1, 64, 2),
                           np.arange(64, 128, 2), np.arange(65, 128, 2)])
    Wk_f = Wk[perm].astype(np.float32)

    # [p, k, d] layouts: w_d[p, k, d] = W^T[k*128+p, d]
    wk_h = np.ascontiguousarray(
        Wk_f.T.reshape(NK, P, D).astype(NPBF))
    wk_h = np.ascontiguousarray(wk_h.transpose(1, 0, 2))
    wv_h = np.ascontiguousarray(
        Wv.astype(np.float32).T.reshape(NK, P, D).astype(NPBF).transpose(1, 0, 2))

    ii = np.arange(P)[:, None]
    jj = np.arange(P)[None, :]
    dmask = (ii <= jj).astype(NPBF)
    ones = np.ones((P, P), dtype=NPBF)

    # hst_d[tb4, p, k, t] = hs[b][tb4*512+t, k*128+p]
    hst_b = []
    for b in range(B):
        a = hidden_states[b].reshape(TB4, 512, NK, P).transpose(0, 3, 2, 1)
        hst_b.append(np.ascontiguousarray(a.astype(NPBF)))

    wq_q, wo_q = [], []
    for q in range(CPB):
        rows = slice(q * HD_PC, (q + 1) * HD_PC)
        # wq_d[p, k, hd] = Wq_f[q*512+hd, k*128+p]
        a = Wq_f[rows].T.reshape(NK, P, HD_PC).transpose(1, 0, 2)
        wq_q.append(np.ascontiguousarray(a.astype(NPBF)))
        # wo_d[p, h, n] = Wo[n, q*512 + h*128 + p]
        a = Wo[:, rows].T.reshape(HPC, P, HID).transpose(1, 0, 2)
        wo_q.append(np.ascontiguousarray(a.astype(NPBF)))

    in_maps = []
    for c in range(NCORES):
        b, q = c // CPB, c % CPB
        in_maps.append({
            "hst": hst_b[b],
            "wq": wq_q[q],
            "wk": wk_h,
            "wv": wv_h,
            "wo": wo_q[q],
            "dmask": dmask,
            "onesd": ones,
        })
    return in_maps


def kernel(hidden_states, Wq, Wk, Wv, Wo):
    hidden_states = np.asarray(hidden_states, dtype=np.float32)
    Wq = np.asarray(Wq, dtype=np.float32)
    Wk = np.asarray(Wk, dtype=np.float32)
    Wv = np.asarray(Wv, dtype=np.float32)
    Wo = np.asarray(Wo, dtype=np.float32)

    nc = _get_program()
    in_maps = _host_inputs(hidden_states, Wq, Wk, Wv, Wo)
    res = run_bass_kernel_spmd(nc, in_maps, list(range(NCORES)))
    parts = [r["out"].astype(np.float32).reshape(T, HID) for r in res.results]
    out = np.empty((B, T, HID), dtype=np.float32)
    for b in range(B):
        out[b] = parts[CPB * b]
        for q in range(1, CPB):
            out[b] += parts[CPB * b + q]
    return out


# revision 19
# speedup vs baseline: 1.1146x; 1.0014x over previous
"""MQA kernel for Trainium2 (8 NeuronCores, SPMD via bass/Tile).

Problem: nn_MultiQueryAttention (B=2, T=2048, HID=2048, H=16, D=128).

Key algebraic simplification: the reference's apply_rope treats q's layout
as (B,T,H,D) while q is actually (B,H,T,D), so the "position" axis is the
head index -> per-head rotation R_h acting on the D axis only, independent
of sequence position. R_h is folded into Wq on the host. k's rope at pos=0
is a pure channel permutation, folded into Wk. The score scale 1/sqrt(D)
is folded into Wq as well. What remains on-device is a plain causal MQA.

Sharding (uniform SPMD program, per-core data differs):
  core c -> batch c//4, heads (c%4)*4..(c%4)*4+3, full T.
  Each core: Q^T/K^T/V projections, causal softmax attention for its 4
  heads, and a partial out-projection (its heads' rows of Wo^T). The 4
  partials per batch are summed on the host.

Key optimizations vs the first working version (336us -> ~258us):
  * all matmul operands in bf16 (same PE rate as fp32r per the TRN2 cost
    model, half the DMA bytes and LDWEIGHTS traffic); fp32 PSUM accum;
    bf16 output upcast+summed on host. rel err ~3.5e-3 (gate is 2e-2).
  * phase 1 output-block-major with 2 rotating PSUM banks; tb4=0 is
    k-quad-blocked over four separate single-bank PSUM tiles so compute
    tracks DMA arrival (DMA-bandwidth-bound start). V is projected
    directly into its natural [s,d] layout (hst-slice stationary), no PE
    transposes. NOTE: interleaved PSUM accumulation groups are only safe
    across SEPARATE tiles; interleaving groups on slices of ONE tile
    silently corrupts results.
  * softmax denominator matmul uses a full-ones stationary matrix so the
    denominator lands pre-broadcast across all 128 partitions: the
    recip-broadcast matmul and two DVE copies disappear from the per-row
    critical chain.
  * phase 2 runs a 2-deep software pipeline (scores/exp two pairs ahead
    of den/pv; s_ps bufs=3, den bufs=1) so the ~686ns exp latency never
    stalls the PE; diagonal (masked) key tile first in each row.
  * out-projection of the previous row is split into 4 chunks injected
    at pairs 2..5 of the current row, hiding the recip->normalize chain.
  * row order is mostly-descending with tiny rows interspersed, ending
    on a medium row, to minimize the exposed pipeline tail.
  * few large host-pre-laid-out DMA descriptors (~30 total input-side);
    the Sync engine costs ~600ns per descriptor.

A K/V AllGather variant (each core computing T/4 of K/V) was measured:
the 1MB 4-rank AllGather takes ~68us (~15GB/s effective) and erases the
20.5us PE saving; see kernel_v5_ag.py.bak.
"""

import numpy as np
import ml_dtypes
from contextlib import ExitStack

import concourse.bass as bass
import concourse.tile as tile
from concourse import bacc, mybir
from concourse.bass_utils import run_bass_kernel_spmd

F32 = mybir.dt.float32
BF16 = mybir.dt.bfloat16
EXP = mybir.ActivationFunctionType.Exp
NPBF = ml_dtypes.bfloat16

B, T, HID, H, D = 2, 2048, 2048, 16, 128
NCORES = 8
CPB = 4              # cores per batch
HPC = H // CPB       # 4 heads per core
HD_PC = HPC * D      # 512 output dims per core
P = 128
KT = T // P          # 16 key tiles
NK = HID // P        # 16 contraction tiles for projections
TB4 = T // 512       # 4 phase-1 t blocks


def _rope_fold():
    """Per-head rotation matrices R_h (128x128) from the reference's quirky rope."""
    half = D // 2
    theta = 1.0 / (10000.0 ** (np.arange(0, half, 2, dtype=np.float64) / half))
    mats = []
    for h in range(H):
        R = np.zeros((D, D), dtype=np.float64)
        c = np.cos(h * theta)
        s = np.sin(h * theta)
        for j in range(32):
            R[j, 2 * j] = c[j]
            R[j, 2 * j + 1] = -s[j]
            R[32 + j, 2 * j] = s[j]
            R[32 + j, 2 * j + 1] = c[j]
            R[64 + j, 64 + 2 * j] = c[j]
            R[64 + j, 64 + 2 * j + 1] = -s[j]
            R[96 + j, 64 + 2 * j] = s[j]
            R[96 + j, 64 + 2 * j + 1] = c[j]
        mats.append(R)
    return mats


def _build_program():
    nc = bacc.Bacc("TRN2", target_bir_lowering=False, debug=False,
                   enable_asserts=False, num_devices=NCORES)

    hst_d = nc.dram_tensor("hst", [TB4, P, NK, 512], BF16, kind="ExternalInput").ap()
    wq_d = nc.dram_tensor("wq", [P, NK, HD_PC], BF16, kind="ExternalInput").ap()
    wk_d = nc.dram_tensor("wk", [P, NK, D], BF16, kind="ExternalInput").ap()
    wv_d = nc.dram_tensor("wv", [P, NK, D], BF16, kind="ExternalInput").ap()
    wo_d = nc.dram_tensor("wo", [P, HPC, HID], BF16, kind="ExternalInput").ap()
    msk_d = nc.dram_tensor("dmask", [P, P], BF16, kind="ExternalInput").ap()
    ones_d = nc.dram_tensor("onesd", [P, P], BF16, kind="ExternalInput").ap()
    out_d = nc.dram_tensor("out", [KT, P, HID], BF16, kind="ExternalOutput").ap()

    def mm(ps, lhsT, rhs, start, stop):
        nc.tensor.matmul(ps, lhsT=lhsT, rhs=rhs, start=start, stop=stop)

    with tile.TileContext(nc) as tc, ExitStack() as ctx:
        singles = ctx.enter_context(tc.tile_pool(name="singles", bufs=1))
        epool = ctx.enter_context(tc.tile_pool(name="etile", bufs=8))
        rpool = ctx.enter_context(tc.tile_pool(name="small", bufs=2))
        opool = ctx.enter_context(tc.tile_pool(name="outt", bufs=2))

        msk = singles.tile([P, P], BF16)
        ones = singles.tile([P, P], BF16)

        wq_sb = singles.tile([P, NK, HD_PC], BF16)
        wk_sb = singles.tile([P, NK, D], BF16)
        wv_sb = singles.tile([P, NK, D], BF16)
        wo_sb = singles.tile([P, HPC, HID], BF16)
        hst_sb = singles.tile([P, TB4, NK, 512], BF16)

        qt_sb = singles.tile([P, HPC, T], BF16)      # Q^T per head [d, t]
        kt_sb = singles.tile([P, T], BF16)           # K^T [d, s]
        v_sb = singles.tile([P, KT, D], BF16)        # V natural [s-tile, d]
        at_sb = singles.tile([P, HPC, T], BF16)      # normalized O^T per head

        # ---- DMAs, in consumption order (few, large descriptors) ----
        for kq in range(4):
            ksl = slice(kq * 4, (kq + 1) * 4)
            nc.sync.dma_start(out=wq_sb[:, ksl, :], in_=wq_d[:, ksl, :])
            nc.sync.dma_start(out=hst_sb[:, 0, ksl, :], in_=hst_d[0][:, ksl, :])
            nc.sync.dma_start(out=wk_sb[:, ksl, :], in_=wk_d[:, ksl, :])
            nc.sync.dma_start(out=wv_sb[:, ksl, :], in_=wv_d[:, ksl, :])
        for tb4 in range(1, TB4):
            nc.sync.dma_start(out=hst_sb[:, tb4, :, :], in_=hst_d[tb4])
        nc.sync.dma_start(out=msk, in_=msk_d)
        nc.sync.dma_start(out=ones, in_=ones_d)
        for h in range(HPC):
            nc.sync.dma_start(out=wo_sb[:, h, :], in_=wo_d[:, h, :])

        # ---------------- Phase 1: Q/K/V projections ----------------
        with tc.tile_pool(name="ps1z", bufs=1, space="PSUM") as ps1z, \
             tc.tile_pool(name="ps1", bufs=2, space="PSUM") as ps1, \
             tc.tile_pool(name="ps1v", bufs=1, space="PSUM") as ps1v:
            # tb4 = 0: k-quad-blocked over five separate PSUM tiles (one per
            # output block) so compute tracks the DMA arrival order instead
            # of stalling for the whole 2.5 MB of weights+activations.
            zps = [ps1z.tile([P, 512], F32, tag=f"z{ob}", name=f"z{ob}")
                   for ob in range(5)]
            for kq in range(4):
                for ob in range(5):
                    for k in range(kq * 4, kq * 4 + 4):
                        if ob < HPC:
                            lhsT = wq_sb[:, k, ob * D:(ob + 1) * D]
                        else:
                            lhsT = wk_sb[:, k, :]
                        mm(zps[ob][:], lhsT, hst_sb[:, 0, k, :],
                           k == 0, k == NK - 1)
            for ob in range(HPC):
                nc.vector.tensor_copy(qt_sb[:, ob, 0:512], zps[ob][:])
            nc.vector.tensor_copy(kt_sb[:, 0:512], zps[HPC][:])
            vps = ps1v.tile([P, 4, D], F32, tag="vnat")
            for ti in range(4):
                for k in range(NK):
                    mm(vps[:, ti, :], hst_sb[:, 0, k, ti * P:(ti + 1) * P],
                       wv_sb[:, k, :], k == 0, k == NK - 1)
            nc.vector.tensor_copy(v_sb[:, 0:4, :], vps[:])

            for tb4 in range(1, TB4):
                tsl = slice(tb4 * 512, (tb4 + 1) * 512)
                for ob in range(5):          # q heads 0..3, then k
                    ps = ps1.tile([P, 512], F32, tag="proj")
                    for k in range(NK):
                        if ob < HPC:
                            lhsT = wq_sb[:, k, ob * D:(ob + 1) * D]
                        else:
                            lhsT = wk_sb[:, k, :]
                        mm(ps[:], lhsT, hst_sb[:, tb4, k, :], k == 0, k == NK - 1)
                    if ob < HPC:
                        nc.vector.tensor_copy(qt_sb[:, ob, tsl], ps[:])
                    else:
                        nc.vector.tensor_copy(kt_sb[:, tsl], ps[:])
                # V in natural [t, d] layout: hst-slice stationary
                vps = ps1v.tile([P, 4, D], F32, tag="vnat")
                for ti in range(4):
                    for k in range(NK):
                        mm(vps[:, ti, :],
                           hst_sb[:, tb4, k, ti * P:(ti + 1) * P],
                           wv_sb[:, k, :], k == 0, k == NK - 1)
                nc.vector.tensor_copy(v_sb[:, tb4 * 4:(tb4 + 1) * 4, :], vps[:])

        # -------- Phase 2+3: causal attention + fused out-projection ----
        msk_b = msk[:, None, :].to_broadcast([P, HPC, P])

        with tc.tile_pool(name="ps2s", bufs=3, space="PSUM") as ps2s, \
             tc.tile_pool(name="ps2o", bufs=2, space="PSUM") as ps2o, \
             tc.tile_pool(name="ps2d", bufs=1, space="PSUM") as ps2d, \
             tc.tile_pool(name="ps3", bufs=2, space="PSUM") as ps3:

            def op_chunk(tb, jb):
                # one quarter of row tb's out-projection + cast + DMA out
                tsl = slice(tb * P, (tb + 1) * P)
                jsl = slice(jb * 512, (jb + 1) * 512)
                if jb == 0:
                    op_chunk.ost = opool.tile([P, HID], BF16, tag="ost")
                op_ps = ps3.tile([P, 512], F32, tag="op")
                for h in range(HPC):
                    mm(op_ps[:], at_sb[:, h, tsl], wo_sb[:, h, jsl],
                       h == 0, h == HPC - 1)
                nc.vector.tensor_copy(op_chunk.ost[:, jsl], op_ps[:])
                nc.sync.dma_start(out=out_d[tb][:, jsl], in_=op_chunk.ost[:, jsl])

            # Mostly-descending row order with the tiny rows (few key tiles)
            # interspersed mid-stream: their recip chains hide under big
            # rows, and the kernel ends on a medium row instead of a chain
            # of tiny rows with nothing left to overlap.
            rows = [15, 14, 13, 12, 0, 11, 1, 10, 2, 9, 3, 8, 7, 6, 5, 4]
            for ri, tb in enumerate(rows):
                tsl = slice(tb * P, (tb + 1) * P)
                qrhs = qt_sb[:, :, tsl]              # [128, 4, 128]
                den_ps = ps2d.tile([P, HPC, P], F32, tag="den")
                ot_ps = ps2o.tile([P, HPC, P], F32, tag="ot")
                sts = [tb] + list(range(tb))         # diagonal tile first
                n = len(sts)
                e_tiles = {}

                def sc_exp(j):
                    s_ps = ps2s.tile([P, HPC, P], F32, tag="sps")
                    mm(s_ps[:], kt_sb[:, sts[j] * P:(sts[j] + 1) * P], qrhs,
                       True, True)
                    e_sb = epool.tile([P, HPC, P], BF16, tag="etile")
                    nc.scalar.activation(e_sb[:], s_ps[:], EXP)
                    if j == 0:  # diagonal tile: causal mask
                        nc.vector.tensor_mul(e_sb[:], e_sb[:], msk_b)
                    e_tiles[j] = e_sb

                # 2-deep software pipeline: scores/exp run 2 pairs ahead of
                # den/pv so the exp latency never stalls the PE.
                opq = [] if ri == 0 else [0, 1, 2, 3]
                prev_tb = rows[ri - 1] if ri > 0 else None
                for j in range(min(2, n)):
                    sc_exp(j)
                for i in range(n):
                    if i + 2 < n:
                        sc_exp(i + 2)
                    e_sb = e_tiles.pop(i)
                    st_, sp_ = (i == 0), (i == n - 1)
                    if i == 0:
                        # pv first: den bank (bufs=1) may still be read by the
                        # previous row's reciprocal — buy it one matmul of slack
                        mm(ot_ps[:], v_sb[:, sts[i], :], e_sb[:], st_, sp_)
                        mm(den_ps[:], ones[:, :], e_sb[:], st_, sp_)
                    else:
                        mm(den_ps[:], ones[:, :], e_sb[:], st_, sp_)
                        mm(ot_ps[:], v_sb[:, sts[i], :], e_sb[:], st_, sp_)
                    if opq and 2 <= i:
                        op_chunk(prev_tb, opq.pop(0))
                while opq:
                    op_chunk(prev_tb, opq.pop(0))
                recip = rpool.tile([P, HPC, P], F32, tag="recip")
                nc.vector.reciprocal_approx_fast(out=recip[:], in_=den_ps[:])
                nc.vector.tensor_mul(at_sb[:, :, tsl], ot_ps[:], recip[:])
            for jb in range(4):
                op_chunk(rows[-1], jb)

    nc.compile()
    return nc


_CACHE = {}


def _get_program():
    if "nc" not in _CACHE:
        _CACHE["nc"] = _build_program()
    return _CACHE["nc"]


def _host_inputs(hidden_states, Wq, Wk, Wv, Wo):
    """Fold rope+scale into weights, build per-core bf16 input maps."""
    f64 = np.float64
    mats = _rope_fold()
    scale = D ** -0.5
    Wq_f = np.empty((HID, HID), dtype=np.float32)
    for h in range(H):
        Wq_f[h * D:(h + 1) * D] = (mats[h] @ Wq[h * D:(h + 1) * D].astype(f64)
                                   * scale).astype(np.float32)
    perm = np.concatenate([np.arange(0, 64, 2), np.arange(1, 64, 2),
                           np.arange(64, 128, 2), np.arange(65, 128, 2)])
    Wk_f = Wk[perm].astype(np.float32)

    # [p, k, d] layouts: w_d[p, k, d] = W^T[k*128+p, d]
    wk_h = np.ascontiguousarray(
        Wk_f.T.reshape(NK, P, D).astype(NPBF))
    wk_h = np.ascontiguousarray(wk_h.transpose(1, 0, 2))
    wv_h = np.ascontiguousarray(
        Wv.astype(np.float32).T.reshape(NK, P, D).astype(NPBF).transpose(1, 0, 2))

    ii = np.arange(P)[:, None]
    jj = np.arange(P)[None, :]
    dmask = (ii <= jj).astype(NPBF)
    ones = np.ones((P, P), dtype=NPBF)

    # hst_d[tb4, p, k, t] = hs[b][tb4*512+t, k*128+p]
    hst_b = []
    for b in range(B):
        a = hidden_states[b].reshape(TB4, 512, NK, P).transpose(0, 3, 2, 1)
        hst_b.append(np.ascontiguousarray(a.astype(NPBF)))

    wq_q, wo_q = [], []
    for q in range(CPB):
        rows = slice(q * HD_PC, (q + 1) * HD_PC)
        # wq_d[p, k, hd] = Wq_f[q*512+hd, k*128+p]
        a = Wq_f[rows].T.reshape(NK, P, HD_PC).transpose(1, 0, 2)
        wq_q.append(np.ascontiguousarray(a.astype(NPBF)))
        # wo_d[p, h, n] = Wo[n, q*512 + h*128 + p]
        a = Wo[:, rows].T.reshape(HPC, P, HID).transpose(1, 0, 2)
        wo_q.append(np.ascontiguousarray(a.astype(NPBF)))

    in_maps = []
    for c in range(NCORES):
        b, q = c // CPB, c % CPB
        in_maps.append({
            "hst": hst_b[b],
            "wq": wq_q[q],
            "wk": wk_h,
            "wv": wv_h,
            "wo": wo_q[q],
            "dmask": dmask,
            "onesd": ones,
        })
    return in_maps


def kernel(hidden_states, Wq, Wk, Wv, Wo):
    hidden_states = np.asarray(hidden_states, dtype=np.float32)
    Wq = np.asarray(Wq, dtype=np.float32)
    Wk = np.asarray(Wk, dtype=np.float32)
    Wv = np.asarray(Wv, dtype=np.float32)
    Wo = np.asarray(Wo, dtype=np.float32)

    nc = _get_program()
    in_maps = _host_inputs(hidden_states, Wq, Wk, Wv, Wo)
    res = run_bass_kernel_spmd(nc, in_maps, list(range(NCORES)))
    parts = [r["out"].astype(np.float32).reshape(T, HID) for r in res.results]
    out = np.empty((B, T, HID), dtype=np.float32)
    for b in range(B):
        out[b] = parts[CPB * b]
        for q in range(1, CPB):
            out[b] += parts[CPB * b + q]
    return out


# revision 21
# speedup vs baseline: 1.1211x; 1.0058x over previous
"""MQA kernel for Trainium2 (8 NeuronCores, SPMD via bass/Tile).

Problem: nn_MultiQueryAttention (B=2, T=2048, HID=2048, H=16, D=128).

Key algebraic simplification: the reference's apply_rope treats q's layout
as (B,T,H,D) while q is actually (B,H,T,D), so the "position" axis is the
head index -> per-head rotation R_h acting on the D axis only, independent
of sequence position. R_h is folded into Wq on the host. k's rope at pos=0
is a pure channel permutation, folded into Wk. The score scale 1/sqrt(D)
is folded into Wq as well. What remains on-device is a plain causal MQA.

Sharding (uniform SPMD program, per-core data differs):
  core c -> batch c//4, heads (c%4)*4..(c%4)*4+3, full T.
  Each core: Q^T/K^T/V projections, causal softmax attention for its 4
  heads, and a partial out-projection (its heads' rows of Wo^T). The 4
  partials per batch are summed on the host.

Key optimizations vs the first working version (336us -> ~258us):
  * all matmul operands in bf16 (same PE rate as fp32r per the TRN2 cost
    model, half the DMA bytes and LDWEIGHTS traffic); fp32 PSUM accum;
    bf16 output upcast+summed on host. rel err ~3.5e-3 (gate is 2e-2).
  * phase 1 output-block-major with 2 rotating PSUM banks; tb4=0 is
    k-quad-blocked over four separate single-bank PSUM tiles so compute
    tracks DMA arrival (DMA-bandwidth-bound start). V is projected
    directly into its natural [s,d] layout (hst-slice stationary), no PE
    transposes. NOTE: interleaved PSUM accumulation groups are only safe
    across SEPARATE tiles; interleaving groups on slices of ONE tile
    silently corrupts results.
  * softmax denominator matmul uses a full-ones stationary matrix so the
    denominator lands pre-broadcast across all 128 partitions: the
    recip-broadcast matmul and two DVE copies disappear from the per-row
    critical chain.
  * phase 2 runs a 2-deep software pipeline (scores/exp two pairs ahead
    of den/pv; s_ps bufs=3, den bufs=1) so the ~686ns exp latency never
    stalls the PE; diagonal (masked) key tile first in each row.
  * out-projection of the previous row is split into 4 chunks injected
    at pairs 2..5 of the current row, hiding the recip->normalize chain.
  * row order is mostly-descending with tiny rows interspersed, ending
    on a medium row, to minimize the exposed pipeline tail.
  * few large host-pre-laid-out DMA descriptors (~30 total input-side);
    the Sync engine costs ~600ns per descriptor.

A K/V AllGather variant (each core computing T/4 of K/V) was measured:
the 1MB 4-rank AllGather takes ~68us (~15GB/s effective) and erases the
20.5us PE saving; see kernel_v5_ag.py.bak.
"""

import numpy as np
import ml_dtypes
from contextlib import ExitStack

import concourse.bass as bass
import concourse.tile as tile
from concourse import bacc, mybir
from concourse.bass_utils import run_bass_kernel_spmd

F32 = mybir.dt.float32
BF16 = mybir.dt.bfloat16
EXP = mybir.ActivationFunctionType.Exp
NPBF = ml_dtypes.bfloat16

B, T, HID, H, D = 2, 2048, 2048, 16, 128
NCORES = 8
CPB = 4              # cores per batch
HPC = H // CPB       # 4 heads per core
HD_PC = HPC * D      # 512 output dims per core
P = 128
KT = T // P          # 16 key tiles
NK = HID // P        # 16 contraction tiles for projections
TB4 = T // 512       # 4 phase-1 t blocks


def _rope_fold():
    """Per-head rotation matrices R_h (128x128) from the reference's quirky rope."""
    half = D // 2
    theta = 1.0 / (10000.0 ** (np.arange(0, half, 2, dtype=np.float64) / half))
    mats = []
    for h in range(H):
        R = np.zeros((D, D), dtype=np.float64)
        c = np.cos(h * theta)
        s = np.sin(h * theta)
        for j in range(32):
            R[j, 2 * j] = c[j]
            R[j, 2 * j + 1] = -s[j]
            R[32 + j, 2 * j] = s[j]
            R[32 + j, 2 * j + 1] = c[j]
            R[64 + j, 64 + 2 * j] = c[j]
            R[64 + j, 64 + 2 * j + 1] = -s[j]
            R[96 + j, 64 + 2 * j] = s[j]
            R[96 + j, 64 + 2 * j + 1] = c[j]
        mats.append(R)
    return mats


def _build_program():
    nc = bacc.Bacc("TRN2", target_bir_lowering=False, debug=False,
                   enable_asserts=False, num_devices=NCORES)

    hst_d = nc.dram_tensor("hst", [TB4, P, NK, 512], BF16, kind="ExternalInput").ap()
    wq_d = nc.dram_tensor("wq", [P, NK, HD_PC], BF16, kind="ExternalInput").ap()
    wk_d = nc.dram_tensor("wk", [P, NK, D], BF16, kind="ExternalInput").ap()
    wv_d = nc.dram_tensor("wv", [P, NK, D], BF16, kind="ExternalInput").ap()
    wo_d = nc.dram_tensor("wo", [P, HPC, HID], BF16, kind="ExternalInput").ap()
    msk_d = nc.dram_tensor("dmask", [P, P], BF16, kind="ExternalInput").ap()
    ones_d = nc.dram_tensor("onesd", [P, P], BF16, kind="ExternalInput").ap()
    out_d = nc.dram_tensor("out", [KT, P, HID], BF16, kind="ExternalOutput").ap()

    def mm(ps, lhsT, rhs, start, stop):
        nc.tensor.matmul(ps, lhsT=lhsT, rhs=rhs, start=start, stop=stop)

    with tile.TileContext(nc) as tc, ExitStack() as ctx:
        singles = ctx.enter_context(tc.tile_pool(name="singles", bufs=1))
        epool = ctx.enter_context(tc.tile_pool(name="etile", bufs=8))
        rpool = ctx.enter_context(tc.tile_pool(name="small", bufs=2))
        opool = ctx.enter_context(tc.tile_pool(name="outt", bufs=2))

        msk = singles.tile([P, P], BF16)
        ones = singles.tile([P, P], BF16)

        wq_sb = singles.tile([P, NK, HD_PC], BF16)
        wk_sb = singles.tile([P, NK, D], BF16)
        wv_sb = singles.tile([P, NK, D], BF16)
        wo_sb = singles.tile([P, HPC, HID], BF16)
        hst_sb = singles.tile([P, TB4, NK, 512], BF16)

        qt_sb = singles.tile([P, HPC, T], BF16)      # Q^T per head [d, t]
        kt_sb = singles.tile([P, T], BF16)           # K^T [d, s]
        v_sb = singles.tile([P, KT, D], BF16)        # V natural [s-tile, d]
        at_sb = singles.tile([P, HPC, T], BF16)      # normalized O^T per head

        # ---- DMAs, in consumption order (few, large descriptors) ----
        for kq in range(4):
            ksl = slice(kq * 4, (kq + 1) * 4)
            nc.sync.dma_start(out=wq_sb[:, ksl, :], in_=wq_d[:, ksl, :])
            nc.sync.dma_start(out=hst_sb[:, 0, ksl, :], in_=hst_d[0][:, ksl, :])
        # wk/wv are first consumed ~17us in (k-projection is the 5th output
        # block); keep them out of the startup-critical DMA window
        nc.sync.dma_start(out=wk_sb, in_=wk_d)
        nc.sync.dma_start(out=wv_sb, in_=wv_d)
        for tb4 in range(1, TB4):
            nc.sync.dma_start(out=hst_sb[:, tb4, :, :], in_=hst_d[tb4])
        nc.sync.dma_start(out=msk, in_=msk_d)
        nc.sync.dma_start(out=ones, in_=ones_d)
        for h in range(HPC):
            nc.sync.dma_start(out=wo_sb[:, h, :], in_=wo_d[:, h, :])

        # ---------------- Phase 1: Q/K/V projections ----------------
        with tc.tile_pool(name="ps1z", bufs=1, space="PSUM") as ps1z, \
             tc.tile_pool(name="ps1", bufs=2, space="PSUM") as ps1, \
             tc.tile_pool(name="ps1v", bufs=1, space="PSUM") as ps1v:
            # tb4 = 0: k-quad-blocked over five separate PSUM tiles (one per
            # output block) so compute tracks the DMA arrival order instead
            # of stalling for the whole 2.5 MB of weights+activations.
            zps = [ps1z.tile([P, 512], F32, tag=f"z{ob}", name=f"z{ob}")
                   for ob in range(5)]
            for kq in range(4):
                for ob in range(5):
                    for k in range(kq * 4, kq * 4 + 4):
                        if ob < HPC:
                            lhsT = wq_sb[:, k, ob * D:(ob + 1) * D]
                        else:
                            lhsT = wk_sb[:, k, :]
                        mm(zps[ob][:], lhsT, hst_sb[:, 0, k, :],
                           k == 0, k == NK - 1)
            for ob in range(HPC):
                nc.vector.tensor_copy(qt_sb[:, ob, 0:512], zps[ob][:])
            nc.vector.tensor_copy(kt_sb[:, 0:512], zps[HPC][:])
            vps = ps1v.tile([P, 4, D], F32, tag="vnat")
            for ti in range(4):
                for k in range(NK):
                    mm(vps[:, ti, :], hst_sb[:, 0, k, ti * P:(ti + 1) * P],
                       wv_sb[:, k, :], k == 0, k == NK - 1)
            nc.vector.tensor_copy(v_sb[:, 0:4, :], vps[:])

            for tb4 in range(1, TB4):
                tsl = slice(tb4 * 512, (tb4 + 1) * 512)
                for ob in range(5):          # q heads 0..3, then k
                    ps = ps1.tile([P, 512], F32, tag="proj")
                    for k in range(NK):
                        if ob < HPC:
                            lhsT = wq_sb[:, k, ob * D:(ob + 1) * D]
                        else:
                            lhsT = wk_sb[:, k, :]
                        mm(ps[:], lhsT, hst_sb[:, tb4, k, :], k == 0, k == NK - 1)
                    if ob < HPC:
                        nc.vector.tensor_copy(qt_sb[:, ob, tsl], ps[:])
                    else:
                        nc.vector.tensor_copy(kt_sb[:, tsl], ps[:])
                # V in natural [t, d] layout: hst-slice stationary
                vps = ps1v.tile([P, 4, D], F32, tag="vnat")
                for ti in range(4):
                    for k in range(NK):
                        mm(vps[:, ti, :],
                           hst_sb[:, tb4, k, ti * P:(ti + 1) * P],
                           wv_sb[:, k, :], k == 0, k == NK - 1)
                nc.vector.tensor_copy(v_sb[:, tb4 * 4:(tb4 + 1) * 4, :], vps[:])

        # -------- Phase 2+3: causal attention + fused out-projection ----
        msk_b = msk[:, None, :].to_broadcast([P, HPC, P])

        with tc.tile_pool(name="ps2s", bufs=3, space="PSUM") as ps2s, \
             tc.tile_pool(name="ps2o", bufs=2, space="PSUM") as ps2o, \
             tc.tile_pool(name="ps2d", bufs=1, space="PSUM") as ps2d, \
             tc.tile_pool(name="ps3", bufs=2, space="PSUM") as ps3:

            def op_chunk(tb, jb):
                # one quarter of row tb's out-projection + cast + DMA out
                tsl = slice(tb * P, (tb + 1) * P)
                jsl = slice(jb * 512, (jb + 1) * 512)
                if jb == 0:
                    op_chunk.ost = opool.tile([P, HID], BF16, tag="ost")
                op_ps = ps3.tile([P, 512], F32, tag="op")
                for h in range(HPC):
                    mm(op_ps[:], at_sb[:, h, tsl], wo_sb[:, h, jsl],
                       h == 0, h == HPC - 1)
                nc.vector.tensor_copy(op_chunk.ost[:, jsl], op_ps[:])
                nc.sync.dma_start(out=out_d[tb][:, jsl], in_=op_chunk.ost[:, jsl])

            # Mostly-descending row order with the tiny rows (few key tiles)
            # interspersed mid-stream: their recip chains hide under big
            # rows, and the kernel ends on a medium row instead of a chain
            # of tiny rows with nothing left to overlap.
            rows = [15, 14, 13, 12, 0, 11, 1, 10, 2, 9, 3, 8, 7, 6, 5, 4]
            for ri, tb in enumerate(rows):
                tsl = slice(tb * P, (tb + 1) * P)
                qrhs = qt_sb[:, :, tsl]              # [128, 4, 128]
                den_ps = ps2d.tile([P, HPC, P], F32, tag="den")
                ot_ps = ps2o.tile([P, HPC, P], F32, tag="ot")
                sts = [tb] + list(range(tb))         # diagonal tile first
                n = len(sts)
                e_tiles = {}

                def sc_exp(j):
                    s_ps = ps2s.tile([P, HPC, P], F32, tag="sps")
                    mm(s_ps[:], kt_sb[:, sts[j] * P:(sts[j] + 1) * P], qrhs,
                       True, True)
                    e_sb = epool.tile([P, HPC, P], BF16, tag="etile")
                    nc.scalar.activation(e_sb[:], s_ps[:], EXP)
                    if j == 0:  # diagonal tile: causal mask
                        nc.vector.tensor_mul(e_sb[:], e_sb[:], msk_b)
                    e_tiles[j] = e_sb

                # 3-deep software pipeline: scores/exp run 3 pairs ahead of
                # den/pv, covering both the exp latency and the previous
                # row's reciprocal still holding the single den PSUM bank.
                opq = [] if ri == 0 else [0, 1, 2, 3]
                prev_tb = rows[ri - 1] if ri > 0 else None
                for j in range(min(3, n)):
                    sc_exp(j)
                for i in range(n):
                    if i + 3 < n:
                        sc_exp(i + 3)
                    e_sb = e_tiles.pop(i)
                    st_, sp_ = (i == 0), (i == n - 1)
                    if i == 0:
                        # pv first: den bank (bufs=1) may still be read by the
                        # previous row's reciprocal — buy it one matmul of slack
                        mm(ot_ps[:], v_sb[:, sts[i], :], e_sb[:], st_, sp_)
                        mm(den_ps[:], ones[:, :], e_sb[:], st_, sp_)
                    else:
                        mm(den_ps[:], ones[:, :], e_sb[:], st_, sp_)
                        mm(ot_ps[:], v_sb[:, sts[i], :], e_sb[:], st_, sp_)
                    if opq and 2 <= i:
                        op_chunk(prev_tb, opq.pop(0))
                while opq:
                    op_chunk(prev_tb, opq.pop(0))
                recip = rpool.tile([P, HPC, P], F32, tag="recip")
                nc.vector.reciprocal_approx_fast(out=recip[:], in_=den_ps[:])
                nc.vector.tensor_mul(at_sb[:, :, tsl], ot_ps[:], recip[:])
            for jb in range(4):
                op_chunk(rows[-1], jb)

    nc.compile()
    return nc


_CACHE = {}


def _get_program():
    if "nc" not in _CACHE:
        _CACHE["nc"] = _build_program()
    return _CACHE["nc"]


def _host_inputs(hidden_states, Wq, Wk, Wv, Wo):
    """Fold rope+scale into weights, build per-core bf16 input maps."""
    f64 = np.float64
    mats = _rope_fold()
    scale = D ** -0.5
    Wq_f = np.empty((HID, HID), dtype=np.float32)
    for h in range(H):
        Wq_f[h * D:(h + 1) * D] = (mats[h] @ Wq[h * D:(h + 1) * D].astype(f64)
                                   * scale).astype(np.float32)
    perm = np.concatenate([np.arange(0, 64, 2), np.arange(1, 64, 2),
                           np.arange(64, 128, 2), np.arange(65, 128, 2)])
    Wk_f = Wk[perm].astype(np.float32)

    # [p, k, d] layouts: w_d[p, k, d] = W^T[k*128+p, d]
    wk_h = np.ascontiguousarray(
        Wk_f.T.reshape(NK, P, D).astype(NPBF))
    wk_h = np.ascontiguousarray(wk_h.transpose(1, 0, 2))
    wv_h = np.ascontiguousarray(
        Wv.astype(np.float32).T.reshape(NK, P, D).astype(NPBF).transpose(1, 0, 2))

    ii = np.arange(P)[:, None]
    jj = np.arange(P)[None, :]
    dmask = (ii <= jj).astype(NPBF)
    ones = np.ones((P, P), dtype=NPBF)

    # hst_d[tb4, p, k, t] = hs[b][tb4*512+t, k*128+p]
    hst_b = []
    for b in range(B):
        a = hidden_states[b].reshape(TB4, 512, NK, P).transpose(0, 3, 2, 1)
        hst_b.append(np.ascontiguousarray(a.astype(NPBF)))

    wq_q, wo_q = [], []
    for q in range(CPB):
        rows = slice(q * HD_PC, (q + 1) * HD_PC)
        # wq_d[p, k, hd] = Wq_f[q*512+hd, k*128+p]
        a = Wq_f[rows].T.reshape(NK, P, HD_PC).transpose(1, 0, 2)
        wq_q.append(np.ascontiguousarray(a.astype(NPBF)))
        # wo_d[p, h, n] = Wo[n, q*512 + h*128 + p]
        a = Wo[:, rows].T.reshape(HPC, P, HID).transpose(1, 0, 2)
        wo_q.append(np.ascontiguousarray(a.astype(NPBF)))

    in_maps = []
    for c in range(NCORES):
        b, q = c // CPB, c % CPB
        in_maps.append({
            "hst": hst_b[b],
            "wq": wq_q[q],
            "wk": wk_h,
            "wv": wv_h,
            "wo": wo_q[q],
            "dmask": dmask,
            "onesd": ones,
        })
    return in_maps


def kernel(hidden_states, Wq, Wk, Wv, Wo):
    hidden_states = np.asarray(hidden_states, dtype=np.float32)
    Wq = np.asarray(Wq, dtype=np.float32)
    Wk = np.asarray(Wk, dtype=np.float32)
    Wv = np.asarray(Wv, dtype=np.float32)
    Wo = np.asarray(Wo, dtype=np.float32)

    nc = _get_program()
    in_maps = _host_inputs(hidden_states, Wq, Wk, Wv, Wo)
    res = run_bass_kernel_spmd(nc, in_maps, list(range(NCORES)))
    parts = [r["out"].astype(np.float32).reshape(T, HID) for r in res.results]
    out = np.empty((B, T, HID), dtype=np.float32)
    for b in range(B):
        out[b] = parts[CPB * b]
        for q in range(1, CPB):
            out[b] += parts[CPB * b + q]
    return out
